# revision 19
# baseline (speedup 1.0000x reference)
"""Trainium2 Bass kernel for the CosFace-style large-margin FC loss.

Model-parallel over the class dim C across 8 cores (12500 cols each),
embeddings replicated. The all-reduce of the softmax denominator / count /
candidate statistics happens host-side on tiny per-core outputs.

Device pipeline per core (fp8 "hot-row" design, ACT-engine-bound):
  - pcos = fp8(emb_n) @ fp8(ker_n_core): fp8e4m3 DoubleRow matmuls
    (K=256 per matmul, 0.5 cycles/col) over 512-padded column tiles,
    groups of [1,3x8] tiles into 3-bank PSUM regions per row-half.
    No label (-2) correction on device (handled on host).
  - ACT: j2 = bf16(exp(64*pcos - 64*tgt)) over the real columns of each
    group (per-partition bias carries the exact f32 -64*tgt). The exp
    table is preloaded at t=0 via a dummy 1-element activation, and a
    few garbage warmup matmuls start the PE p-state ramp early.
  - DVE: exact per-row count of (j2 > 1) == (x > tgt) and the f32 sum
    of j2 (softmax denominator partial) via two 4x-mode tensor_scalar
    accums per group. That is all the vector engine does.
  - Tiny transposed fp8 matmuls (the same SBUF weight tiles as lhsT, an
    8-hot-row embedding slab as rhs) write x values for the 8 rows with
    the largest tgt across ALL columns into 2 single-bank PSUM regions,
    flushed to SBUF by DVE in two phases and DMA'd out (f32).
  - Host: knows tgt exactly, so it knows which rows can contribute to
    the neg set (rows with tgt >= neg_th; certified: only top-2 tgt
    rows matter, 8 shipped). Computes neg_th / neg stats exactly from
    the hot-row values, far_rank from the exact count, the label-column
    denominator correction, and the final loss/acc.
    Accuracy certified against the reference by sim3.py (rel ~1e-4 vs
    the 2e-2 gate; the only approximation is fp8 matmul noise).
"""

import numpy as np

B, D, C = 256, 512, 100000
M = 8
CS = C // M          # 12500 columns per core
TW = 500             # real cols per tile
TP = 512             # padded tile width
NT = CS // TW        # 25 tiles
GROUPS = [1, 3, 3, 3, 3, 3, 3, 3, 3]
NG = len(GROUPS)
R = 8                # hot rows shipped to host
SCALE = 64.0
MARGIN = 0.4
NCAND = 8            # kept for test.py compat

_CACHE = {}


# --------------------------------------------------------------------------
# Tile-framework workaround: walrus in this container accepts at most ONE
# semaphore wait per instruction; Tile emits several. Split them.
# --------------------------------------------------------------------------
def _install_tile_patch():
    import concourse.mybir as mybir
    from concourse.tile import TileContext, ScopedClock

    if getattr(TileContext, "_wait_split_patched", False):
        return

    def _patched_drain_and_barrier(self, tick_clock, wait_clock):
        nc = self.nc
        probe = nc.sync.nop()
        wait_clock.add_sem_waits(
            probe.ins, ScopedClock({None: tick_clock.global_clock})
        )
        si = probe.ins.sync_info
        waits = list(si.on_wait or []) if si is not None else []
        if si is not None:
            si.on_wait = waits[:1]
        for w in waits[1:]:
            nop = nc.sync.nop()
            nop.ins.sync_info = mybir.SyncInfo(on_wait=[w], on_update=[])
        nc.sync.drain()
        nc.all_engine_barrier()
        popped = nc._tile_sem_poison_stack.pop()
        assert popped is self._sem_poison
        nc.clear_and_free_semaphores(list(self.sems.allocated().values()))
        nc.all_engine_barrier()

    TileContext._drain_and_barrier = _patched_drain_and_barrier
    TileContext._wait_split_patched = True


_split_n = [0]


def _split_multi_waits(nc):
    import concourse.mybir as mybir

    for f in nc.m.functions:
        for bb in f.blocks:
            out = []
            changed = False
            for ins in bb.instructions:
                si = ins.sync_info
                if si is not None and si.on_wait and len(si.on_wait) > 1:
                    waits = list(si.on_wait)
                    for w in waits[:-1]:
                        _split_n[0] += 1
                        nop = mybir.InstNoOp(
                            name=f"WSPLIT-{_split_n[0]}", ins=[], outs=[]
                        )
                        nop.engine = ins.engine
                        nop.sync_info = mybir.SyncInfo(on_wait=[w], on_update=[])
                        out.append(nop)
                    si.on_wait = [waits[-1]]
                    changed = True
                out.append(ins)
            if changed:
                bb.instructions = out


# --------------------------------------------------------------------------
# Device program
# --------------------------------------------------------------------------
def _build(reps=1):
    import concourse.bass as bass
    import concourse.mybir as mybir
    from concourse import tile

    _install_tile_patch()
    F = mybir.ActivationFunctionType
    A = mybir.AluOpType
    f32 = mybir.dt.float32
    bf16 = mybir.dt.bfloat16
    f8 = mybir.dt.float8e4
    DR = mybir.MatmulPerfMode.DoubleRow

    nc = bass.Bass()
    w8 = nc.dram_tensor("w8", [NT, 128, 2, 2, TP], f8, kind="ExternalInput")
    ea8 = nc.dram_tensor("ea8", [128, 2, 2, B], f8, kind="ExternalInput")
    eah8 = nc.dram_tensor("eah8", [128, 2, 2, 16], f8, kind="ExternalInput")
    tgtb = nc.dram_tensor("tgtb", [128, 2], f32, kind="ExternalInput")

    ocnt = nc.dram_tensor("ocnt", [128, 2, NG], f32, kind="ExternalOutput")
    osex = nc.dram_tensor("osex", [128, 2, NG], f32, kind="ExternalOutput")
    oxh = nc.dram_tensor("oxh", [128, NT, 4, R], f32, kind="ExternalOutput")

    with tile.TileContext(nc) as tc:
        with (
            tc.tile_pool(name="cst", bufs=1) as cst,
            tc.tile_pool(name="wp", bufs=2) as wp,
            tc.tile_pool(name="jp", bufs=2) as jp,
            tc.tile_pool(name="pp", bufs=1, space="PSUM") as pp,
            tc.tile_pool(name="pta", bufs=1, space="PSUM") as pta,
            tc.tile_pool(name="ptb", bufs=1, space="PSUM") as ptb,
        ):
            scr = cst.tile([128, 1], f32)
            nc.vector.memset(scr[:], 0.0)
            scr2 = cst.tile([128, 1], bf16)
            nc.scalar.activation(scr2[:], scr[:], F.Exp, scale=1.0)
            # PE p-state warmup: garbage matmuls start the ramp clock early
            # (into the ptr_a bank; real transposed matmuls overwrite it)
            scrf8 = cst.tile([128, 2, 16], f8)
            nc.vector.memset(scrf8[:], 0.0)
            pwp = pta.tile([128, 13, 4, R], f32, tag="ptra")
            for _ in range(4):
                nc.tensor.matmul(pwp[0:16, 0, 0:2, :], scrf8[:], scrf8[:],
                                 start=True, stop=True, perf_mode=DR)

            tgtb_s = cst.tile([128, 2], f32)
            nc.gpsimd.dma_start(tgtb_s[:], tgtb[:])
            ea_s = cst.tile([128, 2, 2, B], f8)
            nc.gpsimd.dma_start(ea_s[:], ea8[:])
            eah_s = cst.tile([128, 2, 2, 16], f8)
            nc.gpsimd.dma_start(eah_s[:], eah8[:])

            cnt_acc = cst.tile([128, 2, NG], f32)
            sex_acc = cst.tile([128, 2, NG], f32)
            cj = [cst.tile([128, 3, TW], bf16, name=f"cj{h}") for h in range(2)]
            xh_s = cst.tile([128, NT, 4, R], f32)

            for _ in range(reps):
                ptr_a = pta.tile([128, 13, 4, R], f32, tag="ptra")
                ptr_b = ptb.tile([128, NT - 13, 4, R], f32, tag="ptrb")
                n0 = 0
                for g, G in enumerate(GROUPS):
                    wt = wp.tile([128, 3, 2, 2, TP], f8, tag="wt")
                    eng = nc.sync
                    eng.dma_start(
                        wt[:, 0:G],
                        w8[n0 : n0 + G].rearrange("n p kb t c -> p n kb t c"),
                    )
                    def trans_mms():
                        for i in range(G):
                            n = n0 + i
                            ptr, off = (ptr_a, 0) if n < 13 else (ptr_b, 13)
                            for b in range(4):
                                for kb in range(2):
                                    nc.tensor.matmul(
                                        ptr[:, n - off, b, :],
                                        wt[:, i, kb, :, b * 128 : (b + 1) * 128],
                                        eah_s[:, kb, :, 0:R],
                                        start=(kb == 0),
                                        stop=(kb == 1),
                                        perf_mode=DR,
                                    )

                    if g > 2:
                        trans_mms()
                    for h in range(2):
                        pg = pp.tile([128, 3, TP], f32, tag=f"pg{h}")
                        for i in range(G):
                            for kb in range(2):
                                nc.tensor.matmul(
                                    pg[:, i, :],
                                    ea_s[:, kb, :, h * 128 : (h + 1) * 128],
                                    wt[:, i, kb, :, :],
                                    start=(kb == 0),
                                    stop=(kb == 1),
                                    perf_mode=DR,
                                )
                        j2 = jp.tile([128, 3, TP], bf16, tag=f"j2{h}")
                        nc.scalar.activation(
                            j2[:, 0:G, 0:TW], pg[:, 0:G, 0:TW], F.Exp,
                            bias=tgtb_s[:, h : h + 1], scale=SCALE,
                        )
                        nc.vector.tensor_scalar(
                            out=cj[h][:, 0:G, :], in0=j2[:, 0:G, 0:TW],
                            scalar1=1.0, scalar2=None, op0=A.is_gt, op1=A.add,
                            accum_out=cnt_acc[:, h, g : g + 1],
                        )
                        nc.vector.tensor_scalar(
                            out=cj[h][:, 0:G, :], in0=j2[:, 0:G, 0:TW],
                            scalar1=1.0, scalar2=None, op0=A.mult, op1=A.add,
                            accum_out=sex_acc[:, h, g : g + 1],
                        )
                    if g <= 2:
                        trans_mms()
                    n0 += G
                    if n0 == 13:
                        nc.vector.tensor_scalar(
                            out=xh_s[:, 0:13], in0=ptr_a[:], scalar1=1.0,
                            scalar2=None, op0=A.mult, op1=A.bypass,
                        )
                        nc.sync.dma_start(oxh[:, 0:13], xh_s[:, 0:13])

                nc.vector.tensor_scalar(
                    out=xh_s[:, 13:NT], in0=ptr_b[:], scalar1=1.0,
                    scalar2=None, op0=A.mult, op1=A.bypass,
                )
                nc.sync.dma_start(oxh[:, 13:NT], xh_s[:, 13:NT])

            nc.gpsimd.dma_start(ocnt[:, :, 0 : NG - 1], cnt_acc[:, :, 0 : NG - 1])
            nc.gpsimd.dma_start(osex[:, :, 0 : NG - 1], sex_acc[:, :, 0 : NG - 1])
            nc.gpsimd.dma_start(ocnt[:, :, NG - 1 : NG], cnt_acc[:, :, NG - 1 : NG])
            nc.gpsimd.dma_start(osex[:, :, NG - 1 : NG], sex_acc[:, :, NG - 1 : NG])

    return nc


def _get_nc(split_waits=False, reps=1):
    key = f"nc{reps}"
    if key not in _CACHE:
        _CACHE[key] = _build(reps)
    if split_waits and not _CACHE.get(f"split{reps}"):
        _split_multi_waits(_CACHE[key])
        _CACHE[f"split{reps}"] = True
    return _CACHE[key]


# --------------------------------------------------------------------------
# Host side
# --------------------------------------------------------------------------
def _prep_inputs(embeddings, label, kernel):
    import ml_dtypes

    f8 = ml_dtypes.float8_e4m3
    emb = np.ascontiguousarray(embeddings, dtype=np.float32)
    lab = np.asarray(label).astype(np.int64)
    ker = np.asarray(kernel, dtype=np.float32)

    emb_n = emb / np.sqrt(np.sum(emb * emb, axis=1, keepdims=True,
                                 dtype=np.float32))
    norm = np.sqrt(np.sum(ker * ker, axis=0, dtype=np.float32))
    tgt = np.einsum("rd,dr->r", emb_n, ker[:, lab] / norm[lab][None, :],
                    dtype=np.float32).astype(np.float32)
    hot = np.argsort(-tgt)[:R].astype(np.int64)

    emb8 = emb_n.astype(f8)
    # ea8[p, kb, t, r] = fp8(emb_n[r, kb*256 + t*128 + p])
    ea8 = np.ascontiguousarray(
        emb8.T.reshape(2, 2, 128, B).transpose(2, 0, 1, 3)
    )
    eah8 = np.zeros((128, 2, 2, 16), f8)
    eah8[:, :, :, :R] = np.ascontiguousarray(
        emb8[hot].T.reshape(2, 2, 128, R).transpose(2, 0, 1, 3)
    )
    tgtb = np.ascontiguousarray(
        (-np.float32(SCALE) * tgt).reshape(2, 128).T
    ).astype(np.float32)

    in_maps = []
    for c in range(M):
        wn = (ker[:, c * CS : (c + 1) * CS]
              / norm[c * CS : (c + 1) * CS][None, :]).astype(f8)
        wpad = np.zeros((D, NT, TP), f8)
        wpad[:, :, :TW] = wn.reshape(D, NT, TW)
        # w8[n, p, kb, t, c] = wpad[kb*256 + t*128 + p, n, c]
        w8 = np.ascontiguousarray(
            wpad.reshape(2, 2, 128, NT, TP).transpose(3, 2, 0, 1, 4)
        )
        in_maps.append(dict(w8=w8, ea8=ea8, eah8=eah8, tgtb=tgtb))
    return in_maps, lab, tgt, hot


def kernel(embeddings, label, kernel):
    from concourse.bass_utils import run_bass_kernel_spmd

    in_maps, lab, tgt, hot = _prep_inputs(embeddings, label, kernel)
    nc = _get_nc(split_waits=True)
    res = run_bass_kernel_spmd(nc, in_maps, list(range(M))).results

    # ---- merge per-core partials (host-side all-reduce) ------------------
    cnt = np.zeros(B, np.float64)
    sexp = np.zeros(B, np.float64)
    vals_l, rows_l = [], []
    # oxh[p, n, b, j]: x value of hot row hot[j] at column c*CS + n*TW + col,
    # col = b*128 + p (pad if >= TW)
    p_i = np.arange(128)[:, None, None, None]
    n_i = np.arange(NT)[None, :, None, None]
    b_i = np.arange(4)[None, None, :, None]
    col_in_tile = b_i * 128 + p_i
    real = np.broadcast_to(col_in_tile < TW, (128, NT, 4, R))
    for c in range(M):
        cnt += res[c]["ocnt"].astype(np.float64).sum(axis=2).T.reshape(-1)
        sexp += res[c]["osex"].astype(np.float64).sum(axis=2).T.reshape(-1)
        xh = np.asarray(res[c]["oxh"]).astype(np.float32)   # [128, NT, 4, R]
        gcol = c * CS + n_i * TW + col_in_tile               # [128, NT, 4, 1]
        gcol = np.broadcast_to(gcol, xh.shape)
        for j in range(R):
            r = hot[j]
            m = real[:, :, :, j] & (xh[:, :, :, j] <= tgt[r]) \
                & (gcol[:, :, :, j] != lab[r])
            vals_l.append(xh[:, :, :, j][m])
            rows_l.append(np.full(int(m.sum()), r, np.int64))
    vals = np.concatenate(vals_l)
    rows = np.concatenate(rows_l)

    far = np.float32(1.0 / (C - 1))
    fr = int(np.ceil(far * np.float32(np.int64(B) * (C - 1) - cnt.sum())))
    fr = max(fr, 1)

    order = np.argsort(-vals)
    k_idx = min(fr - 1, vals.size - 1)
    neg_th = np.float32(vals[order[k_idx]])

    sel = vals > neg_th
    xs, rs = vals[sel], rows[sel]
    neg_sum = np.zeros(B, np.float64)
    np.add.at(neg_sum, rs, xs.astype(np.float64) ** 2)
    times = np.zeros(B, np.float64)
    np.add.at(times, rs[xs > 0], 1.0)
    times = np.maximum(times, 1.0)
    neg_mean = (neg_sum / times).astype(np.float32)

    tgt_m = (tgt - np.float32(MARGIN)
             - (np.float32(1.0) + tgt) * neg_mean).astype(np.float32)
    denom = (sexp.astype(np.float32) - np.float32(1.0)
             + np.exp(np.float32(SCALE) * (tgt_m - tgt))).astype(np.float32)
    logp = np.float32(SCALE) * tgt_m - (np.float32(SCALE) * tgt
                                        + np.log(denom))
    loss = np.float32(-np.mean(logp))
    acc = np.float32(np.mean((cnt == 0).astype(np.float32)))
    return np.asarray(loss), np.asarray(acc)
